# revision 17
# baseline (speedup 1.0000x reference)
"""Trainium2 Bass kernel for nn_BinaryLayer (logic-gate network).

Computes: out[b, o] = OR_t AND_a x_in[b, weights[o, t, a]]
where x_in = [const_true | (x != 0) | ~(x != 0)]  (width 1 + 2*784 = 1569),
plus an or-mask: an (o, t) gate whose 16 indices are all 0 is forced False.

Strategy (8 NeuronCores, tensor-parallel over OUT: 128 outs per core).
The execution path charges a large fixed cost per instruction, so the design
minimizes instruction count:

  Host: bit-pack x across batch into u16 words (pure input re-encoding):
        P[g, f] = sum_i (x[16g+i, f] != 0) << i, replicated into the Q7
        gather layout tblin[p, f, w] = P[4*(p%16)+w, f] plus a const-true
        row at f=784.  Table row order: 0..783 = x_f, 784 = const-true,
        785..1568 = ~x_f, 1569 = const-false; weight indices are remapped
        on host accordingly (masked all-zero gates -> row 1569).
  Device per rep (12 engine instructions + 3 semaphores):
        1. XOR  tbl[785:1570] = tbl[0:785] ^ 0xFFFF        (builds ~x rows)
        2. ap_gather: each Q7 core gathers its 8192 gate-slot rows; idx
           order is a-plane-major (position = a*512 + t*16 + o) so every
           tree level below is a fully contiguous half-vs-half 2D op
        3. 4x bitwise AND (contiguous-halves tree over the 16 AND-slots, u32)
        4. 5x bitwise OR  (contiguous-halves tree over the 32 OR-terms, u32)
        5. DMA out [128, 64] u16 packed result bits
  Host: unpack result bits to bool [B, OUT].
"""

import numpy as np

B, F = 1024, 784
OUT, OR_T, AND_T = 1024, 32, 16
NR = 1570  # table rows
N_CORES = 8

_cache = {}


def _build(reps=1, engine="vector"):
    import concourse.bass as bass  # noqa: F401
    import concourse.mybir as mybir
    import concourse.tile as tile
    from concourse.bacc import Bacc

    u16 = mybir.dt.uint16
    i16 = mybir.dt.int16
    u32 = mybir.dt.uint32
    Alu = mybir.AluOpType

    nc = Bacc("TRN2", target_bir_lowering=False, debug=False, num_devices=N_CORES)
    tblin_t = nc.dram_tensor("tblin", [128, 785, 4], u16, kind="ExternalInput")
    idx_t = nc.dram_tensor("idx", [128, 512], i16, kind="ExternalInput")
    out_t = nc.dram_tensor("out", [128, 64], u16, kind="ExternalOutput")

    with tile.TileContext(nc) as tc:
        with (
            tc.tile_pool(name="main", bufs=1) as pool,
            tc.tile_pool(name="gp", bufs=1) as gpool,
        ):
            eng = getattr(nc, engine)
            tbl = pool.tile([128, NR, 4], u16)
            idx_sb = pool.tile([128, 512], i16)
            nc.sync.dma_start(tbl[:, 0:785, :], tblin_t.ap())
            nc.sync.dma_start(idx_sb[:], idx_t.ap())

            for _rep in range(reps):
                # rows 785..1569 = ~rows 0..784  (so row 1569 = ~const = 0);
                # flat 2D u32 view halves the element count vs u16
                tbl2 = tbl[:].bitcast(u32).rearrange("p r w -> p (r w)")
                eng.tensor_scalar(
                    out=tbl2[:, 1570:3140], in0=tbl2[:, 0:1570],
                    scalar1=0, scalar2=None, op0=Alu.bitwise_not,
                )
                gath = gpool.tile([128, 8192, 2], u32, tag="gath")
                nc.gpsimd.ap_gather(
                    gath[:],
                    tbl[:].bitcast(u32),
                    idx_sb[:],
                    channels=128,
                    num_elems=NR,
                    d=2,
                    num_idxs=8192,
                )
                # idx order is a-plane-major: position = a*512 + t*16 + o.
                # AND-tree over the 16 a-planes: every level is a fully
                # contiguous 2D half-vs-half op (cheapest AP on this path).
                cur = gath[:].rearrange("p i w -> p (i w)")  # [128, 16384] u32
                for lvl, n in enumerate((8192, 4096, 2048, 1024)):
                    nxt = gpool.tile([128, n], u32, tag=f"and{lvl}",
                                     name=f"and{lvl}")
                    eng.tensor_tensor(
                        out=nxt[:], in0=cur[:, 0:n], in1=cur[:, n:2 * n],
                        op=Alu.bitwise_and,
                    )
                    cur = nxt[:]
                # and-result [128, (t=32, o=16, w=2)]: OR-tree over t,
                # again contiguous halves
                for lvl, n in enumerate((512, 256, 128, 64, 32)):
                    # bufs=2 on the last tile: the out-DMA of rep r then
                    # doesn't WAR-block rep r+1's final OR
                    nxt = gpool.tile([128, n], u32, tag=f"or{lvl}",
                                     name=f"or{lvl}",
                                     bufs=2 if n == 32 else None)
                    eng.tensor_tensor(
                        out=nxt[:], in0=cur[:, 0:n], in1=cur[:, n:2 * n],
                        op=Alu.bitwise_or,
                    )
                    cur = nxt[:]
                nc.sync.dma_start(out_t.ap(), cur.bitcast(u16))
    nc.compile()
    return nc


def _wrap16(flat):
    """Flat per-Q7-core idx list -> the Q7 16-partition wrapped layout."""
    k = flat.shape[0]
    return flat.reshape(k // 32, 2, 16).transpose(2, 0, 1).reshape(16, k // 16)


def _host_inputs(x, weights):
    x = np.asarray(x)
    w = np.asarray(weights).astype(np.int64)  # [1024, 32, 16]

    # --- bit-pack x: P[g, f] = sum_i (x[16g+i, f] != 0) << i ---
    xb = (x != 0).reshape(64, 16, F)  # [g, i, f]
    pb = np.packbits(
        xb.transpose(0, 2, 1).reshape(-1, 16), axis=1, bitorder="little"
    )  # [(g f), 2] u8
    P = pb.view("<u2").reshape(64, F)  # [g, f] u16
    # tblin[p, f, j] = P[4*(p%16)+j, f];  row 784 = const-true
    t16 = np.ascontiguousarray(P.reshape(16, 4, F).transpose(0, 2, 1))  # [16, f, j]
    tblin = np.empty((128, 785, 4), np.uint16)
    tblin[:, :F, :] = np.tile(t16, (8, 1, 1))
    tblin[:, F, :] = 0xFFFF

    # --- remap weight indices to the new table row order ---
    # orig 0 (const-true) -> 784; orig 1..784 (x_f) -> f = v-1;
    # orig 785..1568 (~x_f) -> unchanged; masked all-zero gates -> 1569.
    wr = np.where(w == 0, 784, np.where(w <= F, w - 1, w))
    allzero = (w == 0).all(-1)  # [1024, 32]
    wr = np.where(allzero[:, :, None], NR - 1, wr).astype(np.int16)

    idx_maps = []
    for cc in range(N_CORES):
        rows = np.zeros((128, 512), np.int16)
        for c in range(8):
            o_base = 128 * cc + 16 * c
            # position = a*512 + t*16 + o  (a-plane-major)
            flat = wr[o_base : o_base + 16].transpose(2, 1, 0).reshape(-1)
            rows[16 * c : 16 * (c + 1)] = _wrap16(flat)
        idx_maps.append(rows)
    return tblin, idx_maps


def _assemble(results):
    out = np.zeros((B, OUT), dtype=bool)
    for cc in range(N_CORES):
        o16 = np.ascontiguousarray(results[cc]["out"]).view(np.uint16)
        o16 = o16.reshape(128, 16, 4)  # [p=16c+l, o_local, j]
        bits = np.unpackbits(
            o16.astype("<u2").view(np.uint8).reshape(128, 16, 4, 2),
            axis=-1,
            bitorder="little",
        ).reshape(128, 16, 4, 16)  # [p, ol, j, bit]
        a = bits.reshape(8, 16, 16, 4, 16)  # [c, l, ol, j, bit]
        # batch = 64l + 16j + bit ; out col = 128cc + 16c + ol
        blk = a.transpose(1, 3, 4, 0, 2).reshape(B, 128)
        out[:, 128 * cc : 128 * (cc + 1)] = blk.astype(bool)
    return out


def kernel(x, weights):
    from concourse.bass_utils import run_bass_kernel_spmd

    if "nc" not in _cache:
        _cache["nc"] = _build(reps=1)
    nc = _cache["nc"]

    tblin, idx_maps = _host_inputs(x, weights)
    in_maps = [{"tblin": tblin, "idx": idx_maps[cc]} for cc in range(N_CORES)]
    try:
        res = run_bass_kernel_spmd(nc, in_maps, core_ids=list(range(N_CORES)))
    except Exception:
        # transient device/tunnel errors: retry once on a fresh attempt
        res = run_bass_kernel_spmd(nc, in_maps, core_ids=list(range(N_CORES)))
    return _assemble(res.results)
